# revision 24
# baseline (speedup 1.0000x reference)
"""Policy-masked sparse attention on 8 trn2 NeuronCores.

Strategy (data-parallel over B: one batch element per core):
  The reference softmax-with-policy (eps=1e-6) reduces, for this input
  regime, to:
    - dropped queries (policy=0): out row = v_row exactly (rel err ~1e-5)
    - kept queries: out row = (E @ V') / (E @ pol), E = exp(S), over kept
      keys only (diagonal is included since a kept query is a kept key)
  Scores are small (|S| < ~3) so exp needs no row-max subtraction
  (shift-invariance holds once eps is negligible).

  Host side: compact kept/dropped token indices per batch (counts ~700/
  ~320), pad to multiples of 128, repack every input into a few large
  [128, F] buffers so each becomes a single wide DMA, cast the qkv/x/
  proj operands to fp8e4 with contraction blocks pair-interleaved for
  DoubleRow matmuls (2 fp8 weights per PE cell -> 256-deep contraction
  per instruction). Weights are pre-scaled by 32 so fp8 values sit in
  the normal range; the exp activation's free `scale` and a final
  1/1024 tensor_scalar absorb the compensation. The projection bias is
  folded in as an extra ones-row contraction term.

  Device side per core:
    qkv projections and V in fp8 DoubleRow; S^T = K^T.T @ Q^T in fp16
    [key, query] layout (contraction is only 64 deep; the two heads of
    a pair use disjoint 64-row strips of the PE array and are emitted
    back-to-back so the PE can overlap them). exp on ScalarE (scale
    1/8192) writes fp8 E^T pairs -> T^T = [pol|V].T @ E^T DoubleRow
    over key-block pairs in [128, query] PSUM; row 0 is the softmax
    denominator (pol sits at column 0, V at 64-aligned columns):
    reciprocal_approx_fast in place + gpsimd partition_broadcast ->
    tensor_tensor normalize writes fp8 attention output -> fp8
    DoubleRow projection + ones-row bias term -> 1/1024 scale on DVE.
    Dropped tokens get x_d @ (Wproj@Wv)^T in fp16. The last head pair
    is processed in query-column chunks so the tail projection
    pipelines with it. Host scatters rows back.
"""

import math
import numpy as np
import ml_dtypes

import concourse.bass as bass
import concourse.bacc as bacc
import concourse.mybir as mybir
from concourse import tile
from concourse.bass_utils import run_bass_kernel_spmd

C = 768
H = 12
HD = 64
CB = C // 128          # feature blocks of 128
PB = CB // 2           # DoubleRow contraction-block pairs
VW = 128               # per-head t-stride in V_aug: pol | 63 pad | 64 v
                       # (pol at col 0 -> denominator lands on PSUM
                       # partition 0 where the DVE reciprocal can read it
                       # in place; V at 64-aligned cols for legal 64-wide
                       # partition access)
F16 = mybir.dt.float16
F32 = mybir.dt.float32
F8 = mybir.dt.float8e4
DR = mybir.MatmulPerfMode.DoubleRow
WS = 32.0              # fp8 weight pre-scale (keeps values normal-range)

_cache = {}


def _groups(n, limit=512):
    out = []
    off = 0
    while off < n:
        g = min(limit, n - off)
        out.append((off, g))
        off += g
    return out


def _build(NK, ND, NKM):
    """Build + bacc-compile the 8-core SPMD program for padded sizes."""
    KB = NK // 128
    DB = ND // 128
    KP = KB // 2          # key-block pairs for the DoubleRow T matmul
    nc = bacc.Bacc("TRN2", target_bir_lowering=False, debug=False,
                   num_devices=8)

    # host-packed [128, F] buffers: one wide DMA each
    wkT = nc.dram_tensor("wkT", [128, 2 * PB * C], F8, kind="ExternalInput").ap()
    wqT = nc.dram_tensor("wqT", [128, 2 * PB * C], F8, kind="ExternalInput").ap()
    wvT = nc.dram_tensor("wvT", [128, 2 * PB * C], F8, kind="ExternalInput").ap()
    wpT = nc.dram_tensor("wpT", [128, 2 * PB * C], F8, kind="ExternalInput").ap()
    xcT = nc.dram_tensor("xcT", [128, 2 * PB * NK], F8, kind="ExternalInput").ap()
    xdT = nc.dram_tensor("xdT", [128, CB * ND], F16, kind="ExternalInput").ap()
    w2T = nc.dram_tensor("w2T", [128, CB * C], F16, kind="ExternalInput").ap()
    polb = nc.dram_tensor("polb", [128, KB * H], F16, kind="ExternalInput").ap()
    bpk = nc.dram_tensor("bpk", [1, C], F16, kind="ExternalInput").ap()
    bpd = nc.dram_tensor("bpd", [1, C], F16, kind="ExternalInput").ap()
    outk = nc.dram_tensor("outk", [NK, C], F16, kind="ExternalOutput").ap()
    outd = nc.dram_tensor("outd", [ND, C], F16, kind="ExternalOutput").ap()

    GK = _groups(NK)             # moving-dim groups over kept tokens
    GKM = _groups(NKM)           # moving-dim groups over real kept queries
    GC = _groups(C)              # moving-dim groups over features

    with tile.TileContext(nc) as tc:
        with (
            tc.tile_pool(name="const", bufs=1) as cpool,
            tc.tile_pool(name="ins", bufs=1) as ipool,
            tc.tile_pool(name="acts", bufs=1) as apool,
            tc.tile_pool(name="work", bufs=4) as wpool,
            tc.tile_pool(name="outs", bufs=3) as opool,
            tc.tile_pool(name="ps", bufs=3, space="PSUM") as pspool,
            tc.tile_pool(name="pt", bufs=2, space="PSUM") as ptpool,
        ):
            # ---- inputs (DMA emission order = priority order) ----
            wk_t = ipool.tile([128, 2 * PB * C], F8, name="wk", tag="wk")
            wq_t = ipool.tile([128, 2 * PB * C], F8, name="wq", tag="wq")
            wv_t = ipool.tile([128, 2 * PB * C], F8, name="wv", tag="wv")
            wp_t = ipool.tile([128, 2 * PB * C], F8, name="wp", tag="wp")
            xc_t = ipool.tile([128, 2 * PB * NK], F8, name="xc", tag="xc")
            xd_t = ipool.tile([128, CB * ND], F16, name="xd", tag="xd")
            w2_t = ipool.tile([128, CB * C], F16, name="w2", tag="w2")
            pol_t = ipool.tile([128, KB * H], F16, name="pol", tag="pol")
            bpk_t = cpool.tile([1, C], F16, name="bpk", tag="bpk")
            bpd_t = cpool.tile([1, C], F16, name="bpd", tag="bpd")

            # HAM warm-up: ~4us of dummy matmuls on scratch keep the PE
            # busy while the first DMAs stream, so the clock gate is
            # already open (2.4 GHz) when real work starts.
            scr = cpool.tile([128, 512], F16, name="scr", tag="scr")
            nc.gpsimd.memset(scr[:], 0.5)
            wps = ptpool.tile([128, 384], F32, name="warm", tag="t2")
            for _ in range(24):
                nc.tensor.matmul(wps[:, 0:384], lhsT=scr[:, 0:128],
                                 rhs=scr[:, 0:384], start=True, stop=True)

            # K weights + x first (they gate the first qkv chunks and S):
            # pair 0 alone, then the rest, so the first DoubleRow
            # contraction block lands as early as possible.
            for pb in range(PB):
                cs, ce = pb * 2 * C, (pb + 1) * 2 * C
                xs, xe = pb * 2 * NK, (pb + 1) * 2 * NK
                nc.sync.dma_start(wk_t[:, cs:ce], wkT[:, cs:ce])
                nc.sync.dma_start(xc_t[:, xs:xe], xcT[:, xs:xe])
                nc.sync.dma_start(wq_t[:, cs:ce], wqT[:, cs:ce])
            nc.sync.dma_start(pol_t[:], polb[:])
            nc.sync.dma_start(wv_t[:], wvT[:])
            nc.sync.dma_start(xd_t[:], xdT[:])
            nc.sync.dma_start(w2_t[:], w2T[:])
            nc.sync.dma_start(bpk_t[:], bpk[:])
            nc.sync.dma_start(bpd_t[:], bpd[:])
            nc.sync.dma_start(wp_t[:], wpT[:])

            ones_t = cpool.tile([1, NK], F16, name="ones", tag="ones")
            nc.gpsimd.memset(ones_t[:], 1.0)

            # ---- persistent intermediates ----
            QcT = [apool.tile([128, NKM], F16, name=f"q{j}", tag=f"q{j}")
                   for j in range(CB)]
            KcT = [apool.tile([128, NK], F16, name=f"k{j}", tag=f"k{j}")
                   for j in range(CB)]
            # V_aug pairs for DoubleRow T: [128, h, t, VW] flattened
            Vag = [apool.tile([128, H * 2 * VW], F8, name=f"va{kp}",
                              tag=f"va{kp}") for kp in range(KP)]
            # attention output feature-block pairs for DoubleRow proj
            OATP = [apool.tile([128, 2 * NK], F8, name=f"oat{p}",
                               tag=f"oat{p}") for p in range(PB)]
            # E^T key-block pairs for DoubleRow T
            ET = {}
            for hm in range(6):
                for kp in range(KP):
                    ET[(hm, kp)] = apool.tile(
                        [128, 2 * NKM], F8, name=f"et{hm}_{kp}",
                        tag=f"et{hm}_{kp}")

            def xc3(pb):
                return xc_t[:, pb * 2 * NK:(pb + 1) * 2 * NK].rearrange(
                    "p (t n) -> p t n", t=2)

            def w3(w, pb):
                return w[:, pb * 2 * C:(pb + 1) * 2 * C].rearrange(
                    "p (t c) -> p t c", t=2)

            def k_chunk(j, eager=False):
                """f-major chunk j of K^T (needs all NK key columns).
                eager: group-major order + per-group copies so the first
                S matmuls unblock on the first 512 columns."""
                ps = pspool.tile([128, NK], F32, name="kps", tag="s")
                if eager:
                    for (o, n) in GK:
                        for pb in range(PB):
                            nc.tensor.matmul(
                                ps[:, o:o + n],
                                lhsT=w3(wk_t, pb)[:, :, j * 128:(j + 1) * 128],
                                rhs=xc3(pb)[:, :, o:o + n],
                                start=(pb == 0), stop=(pb == PB - 1),
                                perf_mode=DR)
                        nc.vector.tensor_copy(KcT[j][:, o:o + n],
                                              ps[:, o:o + n])
                    return
                for pb in range(PB):
                    for (o, n) in GK:
                        nc.tensor.matmul(
                            ps[:, o:o + n],
                            lhsT=w3(wk_t, pb)[:, :, j * 128:(j + 1) * 128],
                            rhs=xc3(pb)[:, :, o:o + n],
                            start=(pb == 0), stop=(pb == PB - 1),
                            perf_mode=DR)
                nc.vector.tensor_copy(KcT[j][:], ps[:])

            def q_chunk(j, eager=False):
                """f-major chunk j of Q^T (real kept queries only)."""
                ps = pspool.tile([128, NKM], F32, name="qps", tag="s")
                if eager:
                    for (o, n) in GKM:
                        for pb in range(PB):
                            nc.tensor.matmul(
                                ps[:, o:o + n],
                                lhsT=w3(wq_t, pb)[:, :, j * 128:(j + 1) * 128],
                                rhs=xc3(pb)[:, :, o:o + n],
                                start=(pb == 0), stop=(pb == PB - 1),
                                perf_mode=DR)
                        nc.vector.tensor_copy(QcT[j][:, o:o + n],
                                              ps[:, o:o + n])
                    return
                for pb in range(PB):
                    for (o, n) in GKM:
                        nc.tensor.matmul(
                            ps[:, o:o + n],
                            lhsT=w3(wq_t, pb)[:, :, j * 128:(j + 1) * 128],
                            rhs=xc3(pb)[:, :, o:o + n],
                            start=(pb == 0), stop=(pb == PB - 1),
                            perf_mode=DR)
                nc.vector.tensor_copy(QcT[j][:], ps[:])

            def v_chunk(tb):
                """token-major V chunk for kept token block tb."""
                kp, t = tb // 2, tb % 2
                ps = pspool.tile([128, C], F32, name="vps", tag="s")
                for pb in range(PB):
                    for (o, n) in GC:
                        nc.tensor.matmul(
                            ps[:, o:o + n],
                            lhsT=xc3(pb)[:, :, tb * 128:(tb + 1) * 128],
                            rhs=w3(wv_t, pb)[:, :, o:o + n],
                            start=(pb == 0), stop=(pb == PB - 1),
                            perf_mode=DR)
                va3 = Vag[kp][:].rearrange("p (h u) -> p h u", u=2 * VW)
                ps3 = ps[:].rearrange("p (h s) -> p h s", s=HD)
                nc.vector.tensor_copy(va3[:, :, t * VW + 64:t * VW + 64 + HD],
                                      ps3)
                pol3 = pol_t[:, tb * H:(tb + 1) * H].rearrange(
                    "p (h o) -> p h o", o=1)
                nc.vector.tensor_copy(va3[:, :, t * VW:t * VW + 1], pol3)

            def s_pair_kb(p, kb):
                """S^T then exp for both heads of pair p at key block kb.

                The two heads' matmuls go to disjoint 64-row strips of
                the PE array (rows 0-63 vs 64-127) and are emitted
                back-to-back so the hardware can overlap them.
                """
                fc = p
                kp, t = kb // 2, kb % 2
                pss = [pspool.tile([128, NKM], F32, name="sps", tag="s")
                       for _ in range(2)]
                for (o, n) in GKM:
                    for hh in range(2):
                        rows = slice(hh * 64, hh * 64 + 64)
                        nc.tensor.matmul(
                            pss[hh][:, o:o + n],
                            lhsT=KcT[fc][rows, kb * 128:(kb + 1) * 128],
                            rhs=QcT[fc][rows, o:o + n],
                            start=True, stop=True)
                for hh in range(2):
                    h = 2 * p + hh
                    et = ET[(h % 6, kp)]
                    nc.scalar.activation(
                        et[:, t * NKM:(t + 1) * NKM], pss[hh][:],
                        mybir.ActivationFunctionType.Exp,
                        scale=1.0 / (8192.0))

            def t_accum(h, kp, o, w, ptT):
                """T^T DoubleRow accumulation over key-block pair kp."""
                lhsT = Vag[kp][:, h * 2 * VW:(h + 1) * 2 * VW].rearrange(
                    "p (t s) -> p t s", t=2)
                rhs = ET[(h % 6, kp)][:].rearrange(
                    "p (t q) -> p t q", t=2)[:, :, o:o + w]
                nc.tensor.matmul(ptT[:, 0:w], lhsT=lhsT, rhs=rhs,
                                 start=(kp == 0), stop=(kp == KP - 1),
                                 perf_mode=DR)

            def norm_chain(h, ptT, cols):
                """Normalize T^T rows by the denominator row 0 over a
                column slice and write the fp8 attention output."""
                fb = h // 2
                orow = (h % 2) * 64
                w = cols.stop - cols.start
                r_sb = wpool.tile([1, NKM], F32, name="rrow", tag="rrow")
                nc.vector.reciprocal_approx_fast(r_sb[:, 0:w],
                                                 ptT[0:1, 0:w])
                rb = wpool.tile([64, NKM], F32, name="rb", tag="rb")
                nc.gpsimd.partition_broadcast(rb[:, 0:w], r_sb[:, 0:w],
                                              channels=64)
                dst = OATP[fb // 2][orow:orow + 64,
                                    (fb % 2) * NK + cols.start:
                                    (fb % 2) * NK + cols.stop]
                nc.vector.tensor_tensor(dst, ptT[64:128, 0:w],
                                        rb[:, 0:w], op=mybir.AluOpType.mult)

            def proj_kept(tb):
                ps = pspool.tile([128, C], F32, name="pps", tag="s")
                for pb in range(PB):
                    lhsT = OATP[pb][:].rearrange(
                        "p (t n) -> p t n", t=2)[:, :, tb * 128:(tb + 1) * 128]
                    for (o, n) in GC:
                        nc.tensor.matmul(
                            ps[:, o:o + n], lhsT=lhsT,
                            rhs=w3(wp_t, pb)[:, :, o:o + n],
                            start=(pb == 0), stop=False, perf_mode=DR)
                for (o, n) in GC:
                    nc.tensor.matmul(
                        ps[:, o:o + n],
                        lhsT=ones_t[:, tb * 128:(tb + 1) * 128],
                        rhs=bpk_t[:, o:o + n],
                        start=False, stop=True)
                ok = opool.tile([128, C], F16, name="ok", tag="ok")
                nc.vector.tensor_scalar_mul(ok[:], ps[:], 1.0 / (WS * WS))
                nc.sync.dma_start(outk[tb * 128:(tb + 1) * 128, :], ok[:])

            def proj_drop(td):
                ps = pspool.tile([128, C], F32, name="dps", tag="s")
                for cb in range(CB):
                    for (o, n) in GC:
                        nc.tensor.matmul(
                            ps[:, o:o + n],
                            lhsT=xd_t[:, cb * ND + td * 128:cb * ND + (td + 1) * 128],
                            rhs=w2_t[:, cb * C + o:cb * C + o + n],
                            start=(cb == 0), stop=False)
                for (o, n) in GC:
                    nc.tensor.matmul(
                        ps[:, o:o + n],
                        lhsT=ones_t[:, td * 128:(td + 1) * 128],
                        rhs=bpd_t[:, o:o + n],
                        start=False, stop=True)
                ok = opool.tile([128, C], F16, name="ok", tag="ok")
                nc.vector.tensor_copy(ok[:], ps[:])
                nc.sync.dma_start(outd[td * 128:(td + 1) * 128, :], ok[:])

            # ---- schedule ----
            # T^T of pair p-1 is emitted during pair p's S/exp so the PE
            # queue always holds dependency-satisfied work (FIFO engine
            # queues stall on the first waiting instruction).
            NP = H // 2
            TH = _groups(NKM, 384)   # T psum column halves (1 bank each)

            def tpair_units(p):
                """T accumulate + normalize for pair p, split into
                half-width emission units so they interleave into the
                S/exp stream's kb slots and the norm chain of one half
                pipelines with the accumulation of the next."""
                state = {}

                def accum(hh, hi):
                    h = 2 * p + hh
                    o, w = TH[hi]
                    ptT = ptpool.tile([128, 384], F32, name="ptT", tag="t2")
                    state[(hh, hi)] = ptT
                    for kp in range(KP):
                        t_accum(h, kp, o, w, ptT)

                def chain(hh, hi):
                    h = 2 * p + hh
                    o, w = TH[hi]
                    norm_chain(h, state[(hh, hi)], slice(o, o + w))

                units = []
                for hh in range(2):
                    for hi in range(len(TH)):
                        units.append(lambda hh=hh, hi=hi: accum(hh, hi))
                    for hi in range(len(TH)):
                        units.append(lambda hh=hh, hi=hi: chain(hh, hi))
                return units

            def tail_pair(p):
                """Last pair: T + norm in two query-column chunks drawn
                from four distinct PSUM slots, so the kp<KP-1 partial
                accumulations prefetch during the S stream and only the
                final kp matmuls + norm + projection remain after the
                last exp."""
                if NKM > 640:
                    tchunks = [(0, 384), (384, 256), (640, NKM - 640)]
                else:
                    tchunks = _groups(NKM, 384)
                for ci, (o, w) in enumerate(tchunks):
                    for hh in range(2):
                        h = 2 * p + hh
                        if ci == 0:
                            ptT = ptpool.tile([128, 384], F32, name="ptc",
                                              tag="t2")
                        else:
                            ptT = pspool.tile([128, 384], F32, name="pts",
                                              tag="s")
                        for kp in range(KP):
                            t_accum(h, kp, o, w, ptT)
                        norm_chain(h, ptT, slice(o, o + w))
                    lo = (o // 128)
                    hi = min(KB, (o + w) // 128) if ci < len(tchunks) - 1 \
                        else KB
                    for tb in range(lo, hi):
                        proj_kept(tb)

            k_chunk(0, eager=True)
            q_chunk(0, eager=True)
            for p in range(NP):
                # filler units interleave into the kb slots below, so the
                # next pair's S matmuls sit directly behind this pair's in
                # the PE FIFO and the exp stream never starves
                fillers = []
                if p + 1 < NP:
                    fillers.append(lambda j=p + 1: k_chunk(j))
                    fillers.append(lambda j=p + 1: q_chunk(j))
                if p == 0:
                    fillers += [lambda tb=tb: v_chunk(tb) for tb in range(KB)]
                if 1 <= p < 1 + DB:
                    fillers.insert(0, lambda td=p - 1: proj_drop(td))
                if p >= 1:
                    fillers += tpair_units(p - 1)
                for kb in range(KB):
                    s_pair_kb(p, kb)
                    if kb < len(fillers):
                        fillers[kb]()
                for f in fillers[KB:]:
                    f()
            tail_pair(NP - 1)

    nc.compile()
    return nc


def _f8(a):
    return np.clip(a, -240.0, 240.0).astype(ml_dtypes.float8_e4m3)


def kernel(x, policy, Wqkv, Wproj, bproj, _trace=False, _tmpdir=None):
    x = np.asarray(x)
    policy = np.asarray(policy)
    Wqkv = np.asarray(Wqkv, dtype=np.float32)
    Wproj = np.asarray(Wproj, dtype=np.float32)
    bproj = np.asarray(bproj, dtype=np.float32)
    B, N, _ = x.shape
    assert B == 8 and x.shape[2] == C

    pol = policy[:, :, 0] > 0.5
    kept = [np.nonzero(pol[b])[0] for b in range(B)]
    drop = [np.nonzero(~pol[b])[0] for b in range(B)]
    nk = [len(i) for i in kept]
    nd = [len(i) for i in drop]
    NK = max(256, int(math.ceil(max(nk) / 256.0)) * 256)
    ND = max(128, int(math.ceil(max(nd) / 128.0)) * 128)
    NKM = min(NK, max(128, int(math.ceil(max(nk) / 32.0)) * 32))

    key = (NK, ND, NKM)
    if key not in _cache:
        _cache[key] = _build(NK, ND, NKM)
    nc = _cache[key]

    def pack_blocks(mat, w):
        """[C, w] f-major -> [128, CB*w], block cb at cols [cb*w:(cb+1)*w]."""
        return np.ascontiguousarray(
            mat.reshape(CB, 128, w).transpose(1, 0, 2).reshape(128, CB * w))

    def pack_pairs(mat, w):
        """[C, w] f-major -> [128, PB*2*w], DoubleRow pair-interleaved."""
        return np.ascontiguousarray(
            mat.reshape(PB, 2, 128, w).transpose(2, 0, 1, 3).reshape(
                128, PB * 2 * w))

    # shared weight prep (x32 pre-scale keeps fp8 values normal-range)
    wqkvT = Wqkv.T * WS                       # [C, 3C] fp32
    wqTb = pack_pairs(_f8(wqkvT[:, 0:C]), C)
    wkTb = pack_pairs(_f8(wqkvT[:, C:2 * C]), C)
    wvTb = pack_pairs(_f8(wqkvT[:, 2 * C:3 * C]), C)
    wpTb = pack_pairs(_f8(Wproj.T * WS), C)
    W2 = Wproj @ Wqkv[2 * C:3 * C]
    w2Tb = pack_blocks(W2.T.astype(np.float16), C)
    bpk_h = (bproj * WS * WS).astype(np.float16).reshape(1, C)
    bpd_h = bproj.astype(np.float16).reshape(1, C)

    in_maps = []
    for b in range(B):
        xcTa = np.zeros((C, NK), np.float32)
        xcTa[:, :nk[b]] = x[b][kept[b]].T
        xdTa = np.zeros((C, ND), np.float16)
        xdTa[:, :nd[b]] = x[b][drop[b]].T
        polba = np.zeros((128, (NK // 128) * H), np.float16)
        for tb in range(NK // 128):
            rows = min(max(nk[b] - tb * 128, 0), 128)
            polba[:rows, tb * H:(tb + 1) * H] = 1.0
        in_maps.append({
            "xcT": pack_pairs(_f8(xcTa), NK), "xdT": pack_blocks(xdTa, ND),
            "wkT": wkTb, "wqT": wqTb, "wvT": wvTb, "wpT": wpTb,
            "w2T": w2Tb, "polb": polba, "bpk": bpk_h, "bpd": bpd_h,
        })

    res = run_bass_kernel_spmd(nc, in_maps, core_ids=list(range(B)),
                               trace=_trace, tmpdir=_tmpdir)

    out = np.empty((B, N, C), np.float32)
    for b in range(B):
        out[b, kept[b]] = res.results[b]["outk"][:nk[b]].astype(np.float32)
        out[b, drop[b]] = res.results[b]["outd"][:nd[b]].astype(np.float32)
    if _trace:
        kernel._last = res
    return out


# revision 25
# speedup vs baseline: 1.1773x; 1.1773x over previous
"""Policy-masked sparse attention on 8 trn2 NeuronCores.

Strategy (data-parallel over B: one batch element per core):
  The reference softmax-with-policy (eps=1e-6) reduces, for this input
  regime, to:
    - dropped queries (policy=0): out row = v_row exactly (rel err ~1e-5)
    - kept queries: out row = (E @ V') / (E @ pol), E = exp(S), over kept
      keys only (diagonal is included since a kept query is a kept key)
  Scores are small (|S| < ~3) so exp needs no row-max subtraction
  (shift-invariance holds once eps is negligible).

  Host side: compact kept/dropped token indices per batch (counts ~700/
  ~320), pad to multiples of 128, repack every input into a few large
  [128, F] buffers so each becomes a single wide DMA, cast the qkv/x/
  proj operands to fp8e4 with contraction blocks pair-interleaved for
  DoubleRow matmuls (2 fp8 weights per PE cell -> 256-deep contraction
  per instruction). Weights are pre-scaled by 32 so fp8 values sit in
  the normal range; the exp activation's free `scale` and a final
  1/1024 tensor_scalar absorb the compensation. The projection bias is
  folded in as an extra ones-row contraction term.

  Device side per core:
    qkv projections and V in fp8 DoubleRow; S^T = K^T.T @ Q^T in fp16
    [key, query] layout (contraction is only 64 deep; the two heads of
    a pair use disjoint 64-row strips of the PE array and are emitted
    back-to-back so the PE can overlap them). exp on ScalarE (scale
    1/8192) writes fp8 E^T pairs -> T^T = [pol|V].T @ E^T DoubleRow
    over key-block pairs in [128, query] PSUM; row 0 is the softmax
    denominator (pol sits at column 0, V at 64-aligned columns):
    reciprocal_approx_fast in place + gpsimd partition_broadcast ->
    tensor_tensor normalize writes fp8 attention output -> fp8
    DoubleRow projection + ones-row bias term -> 1/1024 scale on DVE.
    Dropped tokens get x_d @ (Wproj@Wv)^T in fp16. The last head pair
    is processed in query-column chunks so the tail projection
    pipelines with it. Host scatters rows back.
"""

import math
import numpy as np
import ml_dtypes

import concourse.bass as bass
import concourse.bacc as bacc
import concourse.mybir as mybir
from concourse import tile
from concourse.bass_utils import run_bass_kernel_spmd

C = 768
H = 12
HD = 64
CB = C // 128          # feature blocks of 128
PB = CB // 2           # DoubleRow contraction-block pairs
VW = 128               # per-head t-stride in V_aug: pol | 63 pad | 64 v
                       # (pol at col 0 -> denominator lands on PSUM
                       # partition 0 where the DVE reciprocal can read it
                       # in place; V at 64-aligned cols for legal 64-wide
                       # partition access)
F16 = mybir.dt.float16
F32 = mybir.dt.float32
F8 = mybir.dt.float8e4
DR = mybir.MatmulPerfMode.DoubleRow
WS = 32.0              # fp8 weight pre-scale (keeps values normal-range)

_cache = {}


def _groups(n, limit=512):
    out = []
    off = 0
    while off < n:
        g = min(limit, n - off)
        out.append((off, g))
        off += g
    return out


def _build(NK, ND, NKM):
    """Build + bacc-compile the 8-core SPMD program for padded sizes."""
    KB = NK // 128
    DB = ND // 128
    KP = KB // 2          # key-block pairs for the DoubleRow T matmul
    nc = bacc.Bacc("TRN2", target_bir_lowering=False, debug=False,
                   num_devices=8)

    # host-packed [128, F] buffers: one wide DMA each
    wkT = nc.dram_tensor("wkT", [128, 2 * PB * C], F8, kind="ExternalInput").ap()
    wqT = nc.dram_tensor("wqT", [128, 2 * PB * C], F8, kind="ExternalInput").ap()
    wvT = nc.dram_tensor("wvT", [128, 2 * PB * C], F8, kind="ExternalInput").ap()
    wpT = nc.dram_tensor("wpT", [128, 2 * PB * C], F8, kind="ExternalInput").ap()
    xcT = nc.dram_tensor("xcT", [128, 2 * PB * NK], F8, kind="ExternalInput").ap()
    xdT = nc.dram_tensor("xdT", [128, CB * ND], F16, kind="ExternalInput").ap()
    w2T = nc.dram_tensor("w2T", [128, CB * C], F16, kind="ExternalInput").ap()
    polb = nc.dram_tensor("polb", [128, KB * H], F16, kind="ExternalInput").ap()
    bpk = nc.dram_tensor("bpk", [1, C], F16, kind="ExternalInput").ap()
    bpd = nc.dram_tensor("bpd", [1, C], F16, kind="ExternalInput").ap()
    outk = nc.dram_tensor("outk", [NK, C], F16, kind="ExternalOutput").ap()
    outd = nc.dram_tensor("outd", [ND, C], F16, kind="ExternalOutput").ap()

    GK = _groups(NK)             # moving-dim groups over kept tokens
    GKM = _groups(NKM)           # moving-dim groups over real kept queries
    GC = _groups(C)              # moving-dim groups over features

    with tile.TileContext(nc) as tc:
        with (
            tc.tile_pool(name="const", bufs=1) as cpool,
            tc.tile_pool(name="ins", bufs=1) as ipool,
            tc.tile_pool(name="acts", bufs=1) as apool,
            tc.tile_pool(name="work", bufs=4) as wpool,
            tc.tile_pool(name="outs", bufs=4) as opool,
            tc.tile_pool(name="ps", bufs=3, space="PSUM") as pspool,
            tc.tile_pool(name="pt", bufs=2, space="PSUM") as ptpool,
        ):
            # ---- inputs (DMA emission order = priority order) ----
            wk_t = ipool.tile([128, 2 * PB * C], F8, name="wk", tag="wk")
            wq_t = ipool.tile([128, 2 * PB * C], F8, name="wq", tag="wq")
            wv_t = ipool.tile([128, 2 * PB * C], F8, name="wv", tag="wv")
            wp_t = ipool.tile([128, 2 * PB * C], F8, name="wp", tag="wp")
            xc_t = ipool.tile([128, 2 * PB * NK], F8, name="xc", tag="xc")
            xd_t = ipool.tile([128, CB * ND], F16, name="xd", tag="xd")
            w2_t = ipool.tile([128, CB * C], F16, name="w2", tag="w2")
            pol_t = ipool.tile([128, KB * H], F16, name="pol", tag="pol")
            bpk_t = cpool.tile([1, C], F16, name="bpk", tag="bpk")
            bpd_t = cpool.tile([1, C], F16, name="bpd", tag="bpd")

            # HAM warm-up: ~4us of dummy matmuls on scratch keep the PE
            # busy while the first DMAs stream, so the clock gate is
            # already open (2.4 GHz) when real work starts.
            scr = cpool.tile([128, 512], F16, name="scr", tag="scr")
            nc.gpsimd.memset(scr[:], 0.5)
            wps = ptpool.tile([128, 384], F32, name="warm", tag="t2")
            for _ in range(24):
                nc.tensor.matmul(wps[:, 0:384], lhsT=scr[:, 0:128],
                                 rhs=scr[:, 0:384], start=True, stop=True)

            # K weights + x first (they gate the first qkv chunks and S):
            # pair 0 alone, then the rest, so the first DoubleRow
            # contraction block lands as early as possible.
            for pb in range(PB):
                cs, ce = pb * 2 * C, (pb + 1) * 2 * C
                xs, xe = pb * 2 * NK, (pb + 1) * 2 * NK
                nc.sync.dma_start(wk_t[:, cs:ce], wkT[:, cs:ce])
                nc.sync.dma_start(xc_t[:, xs:xe], xcT[:, xs:xe])
                nc.sync.dma_start(wq_t[:, cs:ce], wqT[:, cs:ce])
            nc.sync.dma_start(pol_t[:], polb[:])
            nc.sync.dma_start(wv_t[:], wvT[:])
            nc.sync.dma_start(xd_t[:], xdT[:])
            nc.sync.dma_start(w2_t[:], w2T[:])
            nc.sync.dma_start(bpk_t[:], bpk[:])
            nc.sync.dma_start(bpd_t[:], bpd[:])
            nc.sync.dma_start(wp_t[:], wpT[:])

            ones_t = cpool.tile([1, NK], F16, name="ones", tag="ones")
            nc.gpsimd.memset(ones_t[:], 1.0)

            # ---- persistent intermediates ----
            QcT = [apool.tile([128, NKM], F16, name=f"q{j}", tag=f"q{j}")
                   for j in range(CB)]
            KcT = [apool.tile([128, NK], F16, name=f"k{j}", tag=f"k{j}")
                   for j in range(CB)]
            # V_aug pairs for DoubleRow T: [128, h, t, VW] flattened
            Vag = [apool.tile([128, H * 2 * VW], F8, name=f"va{kp}",
                              tag=f"va{kp}") for kp in range(KP)]
            # attention output feature-block pairs for DoubleRow proj
            OATP = [apool.tile([128, 2 * NK], F8, name=f"oat{p}",
                               tag=f"oat{p}") for p in range(PB)]
            # E^T key-block pairs for DoubleRow T
            ET = {}
            for hm in range(6):
                for kp in range(KP):
                    ET[(hm, kp)] = apool.tile(
                        [128, 2 * NKM], F8, name=f"et{hm}_{kp}",
                        tag=f"et{hm}_{kp}")

            def xc3(pb):
                return xc_t[:, pb * 2 * NK:(pb + 1) * 2 * NK].rearrange(
                    "p (t n) -> p t n", t=2)

            def w3(w, pb):
                return w[:, pb * 2 * C:(pb + 1) * 2 * C].rearrange(
                    "p (t c) -> p t c", t=2)

            def k_chunk(j, eager=False):
                """f-major chunk j of K^T (needs all NK key columns).
                eager: group-major order + per-group copies so the first
                S matmuls unblock on the first 512 columns."""
                ps = pspool.tile([128, NK], F32, name="kps", tag="s")
                if eager:
                    for (o, n) in GK:
                        for pb in range(PB):
                            nc.tensor.matmul(
                                ps[:, o:o + n],
                                lhsT=w3(wk_t, pb)[:, :, j * 128:(j + 1) * 128],
                                rhs=xc3(pb)[:, :, o:o + n],
                                start=(pb == 0), stop=(pb == PB - 1),
                                perf_mode=DR)
                        nc.vector.tensor_copy(KcT[j][:, o:o + n],
                                              ps[:, o:o + n])
                    return
                for pb in range(PB):
                    for (o, n) in GK:
                        nc.tensor.matmul(
                            ps[:, o:o + n],
                            lhsT=w3(wk_t, pb)[:, :, j * 128:(j + 1) * 128],
                            rhs=xc3(pb)[:, :, o:o + n],
                            start=(pb == 0), stop=(pb == PB - 1),
                            perf_mode=DR)
                nc.vector.tensor_copy(KcT[j][:], ps[:])

            def q_chunk(j, eager=False):
                """f-major chunk j of Q^T (real kept queries only)."""
                ps = pspool.tile([128, NKM], F32, name="qps", tag="s")
                if eager:
                    for (o, n) in GKM:
                        for pb in range(PB):
                            nc.tensor.matmul(
                                ps[:, o:o + n],
                                lhsT=w3(wq_t, pb)[:, :, j * 128:(j + 1) * 128],
                                rhs=xc3(pb)[:, :, o:o + n],
                                start=(pb == 0), stop=(pb == PB - 1),
                                perf_mode=DR)
                        nc.vector.tensor_copy(QcT[j][:, o:o + n],
                                              ps[:, o:o + n])
                    return
                for pb in range(PB):
                    for (o, n) in GKM:
                        nc.tensor.matmul(
                            ps[:, o:o + n],
                            lhsT=w3(wq_t, pb)[:, :, j * 128:(j + 1) * 128],
                            rhs=xc3(pb)[:, :, o:o + n],
                            start=(pb == 0), stop=(pb == PB - 1),
                            perf_mode=DR)
                nc.vector.tensor_copy(QcT[j][:], ps[:])

            def v_chunk(tb):
                """token-major V chunk for kept token block tb."""
                kp, t = tb // 2, tb % 2
                ps = pspool.tile([128, C], F32, name="vps", tag="s")
                for pb in range(PB):
                    for (o, n) in GC:
                        nc.tensor.matmul(
                            ps[:, o:o + n],
                            lhsT=xc3(pb)[:, :, tb * 128:(tb + 1) * 128],
                            rhs=w3(wv_t, pb)[:, :, o:o + n],
                            start=(pb == 0), stop=(pb == PB - 1),
                            perf_mode=DR)
                va3 = Vag[kp][:].rearrange("p (h u) -> p h u", u=2 * VW)
                ps3 = ps[:].rearrange("p (h s) -> p h s", s=HD)
                nc.vector.tensor_copy(va3[:, :, t * VW + 64:t * VW + 64 + HD],
                                      ps3)
                pol3 = pol_t[:, tb * H:(tb + 1) * H].rearrange(
                    "p (h o) -> p h o", o=1)
                nc.vector.tensor_copy(va3[:, :, t * VW:t * VW + 1], pol3)

            def s_pair_kb(p, kb):
                """S^T then exp for both heads of pair p at key block kb.

                The two heads' matmuls go to disjoint 64-row strips of
                the PE array (rows 0-63 vs 64-127) and are emitted
                back-to-back so the hardware can overlap them.
                """
                fc = p
                kp, t = kb // 2, kb % 2
                pss = [pspool.tile([128, NKM], F32, name="sps", tag="s")
                       for _ in range(2)]
                for (o, n) in GKM:
                    for hh in range(2):
                        rows = slice(hh * 64, hh * 64 + 64)
                        nc.tensor.matmul(
                            pss[hh][:, o:o + n],
                            lhsT=KcT[fc][rows, kb * 128:(kb + 1) * 128],
                            rhs=QcT[fc][rows, o:o + n],
                            start=True, stop=True)
                for hh in range(2):
                    h = 2 * p + hh
                    et = ET[(h % 6, kp)]
                    nc.scalar.activation(
                        et[:, t * NKM:(t + 1) * NKM], pss[hh][:],
                        mybir.ActivationFunctionType.Exp,
                        scale=1.0 / (8192.0))

            def t_accum(h, kp, o, w, ptT):
                """T^T DoubleRow accumulation over key-block pair kp."""
                lhsT = Vag[kp][:, h * 2 * VW:(h + 1) * 2 * VW].rearrange(
                    "p (t s) -> p t s", t=2)
                rhs = ET[(h % 6, kp)][:].rearrange(
                    "p (t q) -> p t q", t=2)[:, :, o:o + w]
                nc.tensor.matmul(ptT[:, 0:w], lhsT=lhsT, rhs=rhs,
                                 start=(kp == 0), stop=(kp == KP - 1),
                                 perf_mode=DR)

            def norm_chain(h, ptT, cols):
                """Normalize T^T rows by the denominator row 0 over a
                column slice and write the fp8 attention output."""
                fb = h // 2
                orow = (h % 2) * 64
                w = cols.stop - cols.start
                r_sb = wpool.tile([1, NKM], F32, name="rrow", tag="rrow")
                nc.vector.reciprocal_approx_fast(r_sb[:, 0:w],
                                                 ptT[0:1, 0:w])
                rb = wpool.tile([64, NKM], F32, name="rb", tag="rb")
                nc.gpsimd.partition_broadcast(rb[:, 0:w], r_sb[:, 0:w],
                                              channels=64)
                dst = OATP[fb // 2][orow:orow + 64,
                                    (fb % 2) * NK + cols.start:
                                    (fb % 2) * NK + cols.stop]
                nc.vector.tensor_tensor(dst, ptT[64:128, 0:w],
                                        rb[:, 0:w], op=mybir.AluOpType.mult)

            def proj_kept(tb):
                ps = pspool.tile([128, C], F32, name="pps", tag="s")
                for pb in range(PB):
                    lhsT = OATP[pb][:].rearrange(
                        "p (t n) -> p t n", t=2)[:, :, tb * 128:(tb + 1) * 128]
                    for (o, n) in GC:
                        nc.tensor.matmul(
                            ps[:, o:o + n], lhsT=lhsT,
                            rhs=w3(wp_t, pb)[:, :, o:o + n],
                            start=(pb == 0), stop=False, perf_mode=DR)
                for (o, n) in GC:
                    nc.tensor.matmul(
                        ps[:, o:o + n],
                        lhsT=ones_t[:, tb * 128:(tb + 1) * 128],
                        rhs=bpk_t[:, o:o + n],
                        start=False, stop=True)
                ok = opool.tile([128, C], F16, name="ok", tag="ok")
                nc.vector.tensor_scalar_mul(ok[:], ps[:], 1.0 / (WS * WS))
                nc.sync.dma_start(outk[tb * 128:(tb + 1) * 128, :], ok[:])

            def proj_drop(td):
                ps = pspool.tile([128, C], F32, name="dps", tag="s")
                for cb in range(CB):
                    for (o, n) in GC:
                        nc.tensor.matmul(
                            ps[:, o:o + n],
                            lhsT=xd_t[:, cb * ND + td * 128:cb * ND + (td + 1) * 128],
                            rhs=w2_t[:, cb * C + o:cb * C + o + n],
                            start=(cb == 0), stop=False)
                for (o, n) in GC:
                    nc.tensor.matmul(
                        ps[:, o:o + n],
                        lhsT=ones_t[:, td * 128:(td + 1) * 128],
                        rhs=bpd_t[:, o:o + n],
                        start=False, stop=True)
                ok = opool.tile([128, C], F16, name="ok", tag="ok")
                nc.vector.tensor_copy(ok[:], ps[:])
                nc.sync.dma_start(outd[td * 128:(td + 1) * 128, :], ok[:])

            # ---- schedule ----
            # T^T of pair p-1 is emitted during pair p's S/exp so the PE
            # queue always holds dependency-satisfied work (FIFO engine
            # queues stall on the first waiting instruction).
            NP = H // 2
            TH = _groups(NKM, 384)   # T psum column halves (1 bank each)

            def tpair_units(p):
                """T accumulate + normalize for pair p, split into
                half-width emission units so they interleave into the
                S/exp stream's kb slots and the norm chain of one half
                pipelines with the accumulation of the next."""
                state = {}

                def accum(hh, hi):
                    h = 2 * p + hh
                    o, w = TH[hi]
                    ptT = ptpool.tile([128, 384], F32, name="ptT", tag="t2")
                    state[(hh, hi)] = ptT
                    for kp in range(KP):
                        t_accum(h, kp, o, w, ptT)

                def chain(hh, hi):
                    h = 2 * p + hh
                    o, w = TH[hi]
                    norm_chain(h, state[(hh, hi)], slice(o, o + w))

                units = []
                for hh in range(2):
                    for hi in range(len(TH)):
                        units.append(lambda hh=hh, hi=hi: accum(hh, hi))
                    for hi in range(len(TH)):
                        units.append(lambda hh=hh, hi=hi: chain(hh, hi))
                return units

            def tail_pair(p):
                """Last pair: T + norm in two query-column chunks drawn
                from four distinct PSUM slots, so the kp<KP-1 partial
                accumulations prefetch during the S stream and only the
                final kp matmuls + norm + projection remain after the
                last exp."""
                if NKM > 640:
                    tchunks = [(0, 384), (384, 256), (640, NKM - 640)]
                else:
                    tchunks = _groups(NKM, 384)
                for ci, (o, w) in enumerate(tchunks):
                    for hh in range(2):
                        h = 2 * p + hh
                        if ci == 0:
                            ptT = ptpool.tile([128, 384], F32, name="ptc",
                                              tag="t2")
                        else:
                            ptT = pspool.tile([128, 384], F32, name="pts",
                                              tag="s")
                        for kp in range(KP):
                            t_accum(h, kp, o, w, ptT)
                        norm_chain(h, ptT, slice(o, o + w))
                    lo = (o // 128)
                    hi = min(KB, (o + w) // 128) if ci < len(tchunks) - 1 \
                        else KB
                    for tb in range(lo, hi):
                        proj_kept(tb)

            k_chunk(0, eager=True)
            q_chunk(0, eager=True)
            for p in range(NP):
                # filler units interleave into the kb slots below, so the
                # next pair's S matmuls sit directly behind this pair's in
                # the PE FIFO and the exp stream never starves
                fillers = []
                if p + 1 < NP:
                    fillers.append(lambda j=p + 1: k_chunk(j))
                    fillers.append(lambda j=p + 1: q_chunk(j))
                if p == 0:
                    fillers += [lambda tb=tb: v_chunk(tb) for tb in range(KB)]
                if 1 <= p < 1 + DB:
                    fillers.insert(0, lambda td=p - 1: proj_drop(td))
                if p >= 1:
                    fillers += tpair_units(p - 1)
                for kb in range(KB):
                    s_pair_kb(p, kb)
                    if kb < len(fillers):
                        fillers[kb]()
                for f in fillers[KB:]:
                    f()
            tail_pair(NP - 1)

    nc.compile()
    return nc


def _f8(a):
    return np.clip(a, -240.0, 240.0).astype(ml_dtypes.float8_e4m3)


def kernel(x, policy, Wqkv, Wproj, bproj, _trace=False, _tmpdir=None):
    x = np.asarray(x)
    policy = np.asarray(policy)
    Wqkv = np.asarray(Wqkv, dtype=np.float32)
    Wproj = np.asarray(Wproj, dtype=np.float32)
    bproj = np.asarray(bproj, dtype=np.float32)
    B, N, _ = x.shape
    assert B == 8 and x.shape[2] == C

    pol = policy[:, :, 0] > 0.5
    kept = [np.nonzero(pol[b])[0] for b in range(B)]
    drop = [np.nonzero(~pol[b])[0] for b in range(B)]
    nk = [len(i) for i in kept]
    nd = [len(i) for i in drop]
    NK = max(256, int(math.ceil(max(nk) / 256.0)) * 256)
    ND = max(128, int(math.ceil(max(nd) / 128.0)) * 128)
    NKM = min(NK, max(128, int(math.ceil(max(nk) / 32.0)) * 32))

    key = (NK, ND, NKM)
    if key not in _cache:
        _cache[key] = _build(NK, ND, NKM)
    nc = _cache[key]

    def pack_blocks(mat, w):
        """[C, w] f-major -> [128, CB*w], block cb at cols [cb*w:(cb+1)*w]."""
        return np.ascontiguousarray(
            mat.reshape(CB, 128, w).transpose(1, 0, 2).reshape(128, CB * w))

    def pack_pairs(mat, w):
        """[C, w] f-major -> [128, PB*2*w], DoubleRow pair-interleaved."""
        return np.ascontiguousarray(
            mat.reshape(PB, 2, 128, w).transpose(2, 0, 1, 3).reshape(
                128, PB * 2 * w))

    # shared weight prep (x32 pre-scale keeps fp8 values normal-range)
    wqkvT = Wqkv.T * WS                       # [C, 3C] fp32
    wqTb = pack_pairs(_f8(wqkvT[:, 0:C]), C)
    wkTb = pack_pairs(_f8(wqkvT[:, C:2 * C]), C)
    wvTb = pack_pairs(_f8(wqkvT[:, 2 * C:3 * C]), C)
    wpTb = pack_pairs(_f8(Wproj.T * WS), C)
    W2 = Wproj @ Wqkv[2 * C:3 * C]
    w2Tb = pack_blocks(W2.T.astype(np.float16), C)
    bpk_h = (bproj * WS * WS).astype(np.float16).reshape(1, C)
    bpd_h = bproj.astype(np.float16).reshape(1, C)

    in_maps = []
    for b in range(B):
        xcTa = np.zeros((C, NK), np.float32)
        xcTa[:, :nk[b]] = x[b][kept[b]].T
        xdTa = np.zeros((C, ND), np.float16)
        xdTa[:, :nd[b]] = x[b][drop[b]].T
        polba = np.zeros((128, (NK // 128) * H), np.float16)
        for tb in range(NK // 128):
            rows = min(max(nk[b] - tb * 128, 0), 128)
            polba[:rows, tb * H:(tb + 1) * H] = 1.0
        in_maps.append({
            "xcT": pack_pairs(_f8(xcTa), NK), "xdT": pack_blocks(xdTa, ND),
            "wkT": wkTb, "wqT": wqTb, "wvT": wvTb, "wpT": wpTb,
            "w2T": w2Tb, "polb": polba, "bpk": bpk_h, "bpd": bpd_h,
        })

    res = run_bass_kernel_spmd(nc, in_maps, core_ids=list(range(B)),
                               trace=_trace, tmpdir=_tmpdir)

    out = np.empty((B, N, C), np.float32)
    for b in range(B):
        out[b, kept[b]] = res.results[b]["outk"][:nk[b]].astype(np.float32)
        out[b, drop[b]] = res.results[b]["outd"][:nd[b]].astype(np.float32)
    if _trace:
        kernel._last = res
    return out
